# revision 1
# baseline (speedup 1.0000x reference)
"""Trainium2 Bass kernel for nn_Depth_prompt (gnn_message_passing).

Data-parallel over batch N=8 across 8 NeuronCores (1 image/core).
Per-core pipeline (all on-chip after the depth/cues loads):
  1. weights = sigmoid(reg_W @ depth + reg_b)       PE matmul (bf16), k-major
     channel permutation o' = k*24+l so later reshuffles are
     partition-contiguous.
  2. S = sum_k weights, r = 1/(S+eps)               PE indicator matmul + DVE
  3. encoder: 3x 3x3 convs as im2col (unfold DMAs) + K-packed matmuls
  4. 7-step per-pixel stencil diffusion on DVE, layout (b*24+l, 18, 66)
     with per-step halo-exchange DMAs; normalization folded in as a
     per-step multiply by r.
  5. decoder: 3 convs -> s (1, 4096)
  6. prompts: hdn[j,p] = gelu(s[p]*u[j] + c[j]) via ACT scale/bias;
     out = hdn.T @ sm_W.T (PE, bf16) + sm_b (DVE add on PSUM evac);
     u/c are host-folded from lmlp/depth-adapter weights (rank-1 collapse
     of the hw x 1 @ 1 x HID matmul).
"""
import sys

sys.path.insert(0, "/opt/trn_rl_repo")

import numpy as np
import ml_dtypes

import concourse.bass as bass
import concourse.tile as tile
from concourse import bacc, mybir
from concourse.bass_utils import run_bass_kernel_spmd

f32 = mybir.dt.float32
bf16 = mybir.dt.bfloat16
AF = mybir.ActivationFunctionType

N, H, W, ED, LD, DEPTH = 8, 64, 64, 768, 24, 4
HID = ED // 2
KK, STEPS, EPS = 9, 7, 1e-5
HW = H * W
NCORES = 8
OC = LD * KK  # 216


def build_nc(gelu=True):
    nc = bacc.Bacc("TRN2", target_bir_lowering=False, debug=False,
                   num_devices=NCORES)
    depth_d = nc.dram_tensor("depth", [ED, HW], f32, kind="ExternalInput").ap()
    cues_d = nc.dram_tensor("cues", [1, HW], f32, kind="ExternalInput").ap()
    regT_d = nc.dram_tensor("p_regT", [ED, OC], bf16, kind="ExternalInput").ap()
    regb_d = nc.dram_tensor("p_regb", [128, 2], f32, kind="ExternalInput").ap()
    ind_d = nc.dram_tensor("p_ind", [OC, LD], bf16, kind="ExternalInput").ap()
    cw0_d = nc.dram_tensor("p_cw0", [KK, LD], bf16, kind="ExternalInput").ap()
    # K-packed conv weights (216, O): row k*24+cin
    cwe1_d = nc.dram_tensor("p_cwe1", [OC, LD], bf16, kind="ExternalInput").ap()
    cwe2_d = nc.dram_tensor("p_cwe2", [OC, LD], bf16, kind="ExternalInput").ap()
    cwd0_d = nc.dram_tensor("p_cwd0", [OC, LD], bf16, kind="ExternalInput").ap()
    cwd1_d = nc.dram_tensor("p_cwd1", [OC, LD], bf16, kind="ExternalInput").ap()
    cwd2_d = nc.dram_tensor("p_cwd2", [OC, 1], bf16, kind="ExternalInput").ap()
    cb_d = nc.dram_tensor("p_cb", [LD, 8], f32, kind="ExternalInput").ap()
    R_d = nc.dram_tensor("p_R", [4 * 7, ED], bf16, kind="ExternalInput").ap()
    out_d = nc.dram_tensor("out", [DEPTH, HW, ED],
                           mybir.dt.float16, kind="ExternalOutput").ap()

    gelu_f = AF.Gelu if gelu else AF.Identity

    from contextlib import ExitStack
    with tile.TileContext(nc) as tc, ExitStack() as es:
        _build_body(nc, tc, es, locals())
    nc.compile()
    return nc


def _build_body(nc, tc, es, d):
    depth_d, cues_d, out_d = d["depth_d"], d["cues_d"], d["out_d"]
    gelu_f = d["gelu_f"]

    from contextlib import ExitStack
    pool_const = es.enter_context(tc.tile_pool(name="const", bufs=1))
    es_mid = es.enter_context(ExitStack())
    es_unf = es.enter_context(ExitStack())
    es_sten = es.enter_context(ExitStack())
    es_conv = es.enter_context(ExitStack())
    es_front = es.enter_context(ExitStack())
    pool_mid = es_mid.enter_context(tc.tile_pool(name="mid", bufs=1))
    pool_unf = es_unf.enter_context(tc.tile_pool(name="unf", bufs=2))
    pool_sten = es_sten.enter_context(tc.tile_pool(name="sten", bufs=2))
    pool_front = es_front.enter_context(tc.tile_pool(name="front", bufs=1))
    pool_dep = es_front.enter_context(tc.tile_pool(name="dep", bufs=3))

    # ---------------- consts ----------------
    regT_t = pool_const.tile([128, 6, OC], bf16)
    for cc in range(6):
        nc.sync.dma_start(regT_t[:, cc, :], d["regT_d"][cc * 128:(cc + 1) * 128, :])
    regb_t = pool_const.tile([128, 2], f32)
    nc.sync.dma_start(regb_t[:], d["regb_d"])
    ind_t = pool_const.tile([128, 2, LD], bf16)
    nc.sync.dma_start(ind_t[:, 0, :], d["ind_d"][0:128, :])
    nc.sync.dma_start(ind_t[0:88, 1, :], d["ind_d"][128:OC, :])
    cw0_t = pool_const.tile([KK, LD], bf16)
    nc.sync.dma_start(cw0_t[:], d["cw0_d"])
    # conv weights: chunk-A (128, 5, 24) + chunk-B (88, 5, 24); cols:
    # 0=enc1 1=enc2 2=dec0 3=dec1 4=dec2(first out col only)
    cwA_t = pool_const.tile([128, 5, LD], bf16)
    cwB_t = pool_const.tile([88, 5, LD], bf16)
    for ci, key in enumerate(["cwe1_d", "cwe2_d", "cwd0_d", "cwd1_d"]):
        nc.sync.dma_start(cwA_t[:, ci, :], d[key][0:128, :])
        nc.sync.dma_start(cwB_t[:, ci, :], d[key][128:OC, :])
    nc.sync.dma_start(cwA_t[:, 4, 0:1], d["cwd2_d"][0:128, :])
    nc.sync.dma_start(cwB_t[:, 4, 0:1], d["cwd2_d"][128:OC, :])
    cb_t = pool_const.tile([LD, 8], f32)
    nc.sync.dma_start(cb_t[:], d["cb_d"])
    R_ts = []
    for _i in range(DEPTH):
        R_i = pool_const.tile([7, ED], bf16, tag=f"R{_i}")
        nc.sync.dma_start(R_i[:], d["R_d"][_i * 7:(_i + 1) * 7, :])
        R_ts.append(R_i)
    s_row = pool_const.tile([1, HW], f32)

    # ---------------- front: weights matmul + sigmoid + k-sum ----------------
    wvA = pool_front.tile([128, HW], bf16)
    wvB = pool_front.tile([88, HW], bf16)
    S_sb = pool_front.tile([LD, HW], f32)

    ppconv = es_conv.enter_context(
        tc.tile_pool(name="ppconv", bufs=2, space="PSUM"))
    ppwA = es_front.enter_context(tc.tile_pool(name="ppwA", bufs=2, space="PSUM"))
    ppwB = es_front.enter_context(tc.tile_pool(name="ppwB", bufs=2, space="PSUM"))
    ppS = es_front.enter_context(tc.tile_pool(name="ppS", bufs=2, space="PSUM"))

    for pc in range(8):
        sl = slice(pc * 512, (pc + 1) * 512)
        psA = ppwA.tile([128, 512], f32, tag="psA")
        psB = ppwB.tile([88, 512], f32, tag="psB")
        for cc in range(6):
            dt_t = pool_dep.tile([128, 512], bf16, tag="dt")
            nc.gpsimd.dma_start(dt_t[:], depth_d[cc * 128:(cc + 1) * 128, sl])
            nc.tensor.matmul(psA[:], regT_t[:, cc, 0:128], dt_t[:],
                             start=(cc == 0), stop=(cc == 5))
            nc.tensor.matmul(psB[:], regT_t[:, cc, 128:OC], dt_t[:],
                             start=(cc == 0), stop=(cc == 5))
        nc.scalar.activation(wvA[:, sl], psA[:], AF.Sigmoid,
                             bias=regb_t[:, 0:1], scale=1.0)
        nc.scalar.activation(wvB[:, sl], psB[:], AF.Sigmoid,
                             bias=regb_t[0:88, 1:2], scale=1.0)
        psS = ppS.tile([LD, 512], f32, tag="psS")
        nc.tensor.matmul(psS[:], ind_t[:, 0, :], wvA[:, sl],
                         start=True, stop=False)
        nc.tensor.matmul(psS[:], ind_t[0:88, 1, :], wvB[:, sl],
                         start=False, stop=True)
        nc.scalar.activation(S_sb[:, sl], psS[:], AF.Identity,
                             bias=cb_t[:, 6:7], scale=1.0)

    # ---------------- conv helpers (im2col unfold + K-packed matmul) -------
    # U66 trick: per tap k copy the CONTIGUOUS flat slice of the padded
    # image starting at (di*66+dj); the conv window for output (r,c) is then
    # U66[o, r, c] with a strided (8, 64)-of-66 matmul rhs view.
    FL = 64 * 66  # 4224

    def unfold(xpad):  # xpad: FLAT (p, 4360) tile
        UA = pool_unf.tile([128, H, 66], bf16, tag="UA")
        UB = pool_unf.tile([88, H, 66], bf16, tag="UB")
        xf = xpad
        uaf = UA[:].rearrange("p a b -> p (a b)")
        ubf = UB[:].rearrange("p a b -> p (a b)")
        for k in range(KK):
            di, dj = k // 3, k % 3
            off = di * 66 + dj
            o0 = k * LD
            eng = nc.sync if k % 2 == 0 else nc.scalar
            if o0 + LD <= 128:
                eng.dma_start(uaf[o0:o0 + LD, :], xf[:, off:off + FL])
            elif o0 >= 128:
                eng.dma_start(ubf[o0 - 128:o0 - 128 + LD, :],
                              xf[:, off:off + FL])
            else:
                nA = 128 - o0
                eng.dma_start(uaf[o0:128, :], xf[0:nA, off:off + FL])
                eng.dma_start(ubf[0:LD - nA, :], xf[nA:LD, off:off + FL])
        return UA, UB

    def conv_packed(U, ci, xout, bias_ap, func, m=LD):
        UA, UB = U
        for pc in range(8):
            sl = slice(pc * 512, (pc + 1) * 512)
            ps = ppconv.tile([LD, 512], f32, tag="pconv")
            nc.tensor.matmul(ps[0:m, :], cwA_t[:, ci, 0:m],
                             UA[:, pc * 8:(pc + 1) * 8, 0:W],
                             start=True, stop=False)
            nc.tensor.matmul(ps[0:m, :], cwB_t[:, ci, 0:m],
                             UB[:, pc * 8:(pc + 1) * 8, 0:W],
                             start=False, stop=True)
            if xout is not None:
                r0 = pc * 8
                nc.scalar.activation(
                    xout[:, 1 + r0:9 + r0, 1:65],
                    ps[:].rearrange("p (r c) -> p r c", r=8), func,
                    bias=bias_ap, scale=1.0)
            else:
                nc.scalar.activation(s_row[:, sl], ps[0:1, :], func,
                                     bias=bias_ap, scale=1.0)

    # ---------------- encoder ----------------
    cpad_f = pool_front.tile([1, 4360], bf16)
    nc.gpsimd.memset(cpad_f[:], 0.0)
    cpad = cpad_f[:, 0:4356].rearrange("p (a b) -> p a b", a=66)
    nc.gpsimd.dma_start(
        cpad[:, 1:65, 1:65],
        cues_d[:].rearrange("o (h w) -> o h w", h=H))
    cu9 = pool_front.tile([KK, H, 66], bf16)
    cpf = cpad_f
    cu9f = cu9[:].rearrange("p a b -> p (a b)")
    for k in range(KK):
        di, dj = k // 3, k % 3
        off = di * 66 + dj
        nc.sync.dma_start(cu9f[k:k + 1, :], cpad_f[:, off:off + 64 * 66])

    eA_f = pool_mid.tile([LD, 4360], bf16)
    eB_f = pool_mid.tile([LD, 4360], bf16)
    nc.gpsimd.memset(eA_f[:], 0.0)
    nc.gpsimd.memset(eB_f[:], 0.0)
    eA = eA_f[:, 0:4356].rearrange("p (a b) -> p a b", a=66)
    eB = eB_f[:, 0:4356].rearrange("p (a b) -> p a b", a=66)

    for rc in range(8):
        ps0 = ppconv.tile([LD, 512], f32, tag="pconv")
        ps0v = ps0[:].rearrange("p (r c) -> p r c", r=8)
        nc.tensor.matmul(ps0v, cw0_t[:], cu9[:, rc * 8:(rc + 1) * 8, 0:W],
                         start=True, stop=True)
        nc.scalar.activation(eA[:, 1 + rc * 8:9 + rc * 8, 1:65], ps0v, AF.Relu,
                             bias=cb_t[:, 0:1], scale=1.0)
    U = unfold(eA_f)
    conv_packed(U, 0, eB, cb_t[:, 1:2], AF.Relu)
    U = unfold(eB_f)
    conv_packed(U, 1, eA, cb_t[:, 2:3], AF.Identity)

    # ---------------- stencil setup ----------------
    x_a = pool_mid.tile([96, 18, 66], bf16)
    x_b = pool_mid.tile([96, 18, 66], bf16)
    nc.gpsimd.memset(x_a[:], 0.0)
    nc.gpsimd.memset(x_b[:], 0.0)
    for b in range(4):
        (nc.sync if b % 2 == 0 else nc.scalar).dma_start(
            x_a[b * LD:(b + 1) * LD, :, :], eA[:, b * 16:b * 16 + 18, :])

    rpre = pool_front.tile([96, 16, W], f32)
    rscr = pool_front.tile([96, 16, W], f32)
    rS = pool_front.tile([96, 16, W], f32)
    rSb = pool_mid.tile([96, 16, W], bf16)
    for b in range(4):
        (nc.sync if b % 2 == 0 else nc.scalar).dma_start(
            rpre[b * LD:(b + 1) * LD, :, :],
            S_sb[:, b * 1024:(b + 1) * 1024].rearrange("p (r c) -> p r c", r=16))
    nc.vector.reciprocal_approx_accurate(rS[:], rpre[:], rscr[:])
    nc.vector.tensor_copy(rSb[:], rS[:])

    wv9 = pool_mid.tile([96, KK, 16, W], bf16)
    _wveng = [nc.sync, nc.scalar]
    _wi = 0
    for k in range(KK):
        o0 = k * LD
        for b in range(4):
            src_sl = slice(b * 1024, (b + 1) * 1024)
            dst = wv9[b * LD:(b + 1) * LD, k, :, :]
            eng = _wveng[_wi % 2]
            _wi += 1
            if o0 + LD <= 128:
                eng.dma_start(
                    dst,
                    wvA[o0:o0 + LD, src_sl].rearrange("p (r c) -> p r c", r=16))
            elif o0 >= 128:
                eng.dma_start(
                    dst,
                    wvB[o0 - 128:o0 - 128 + LD, src_sl].rearrange(
                        "p (r c) -> p r c", r=16))
            else:
                nA = 128 - o0
                eng.dma_start(
                    wv9[b * LD:b * LD + nA, k, :, :],
                    wvA[o0:128, src_sl].rearrange("p (r c) -> p r c", r=16))
                eng.dma_start(
                    wv9[b * LD + nA:(b + 1) * LD, k, :, :],
                    wvB[0:LD - nA, src_sl].rearrange("p (r c) -> p r c", r=16))

    es_front.close()

    # ---------------- stencil ----------------
    korder = [4, 3, 5, 1, 7, 6, 8]   # DVE taps (di=1 first: no halo dep)
    xc, xn = x_a, x_b
    for step in range(STEPS):
        acc = pool_sten.tile([96, 16, W], bf16, tag="acc")
        # gpsimd computes taps 0 and 2 into its own partial
        gacc = pool_sten.tile([96, 16, W], bf16, tag="gacc")
        gtmp = pool_sten.tile([96, 16, W], bf16, tag="gtmp")
        nc.gpsimd.tensor_mul(gacc[:], xc[:, 0:16, 0:W], wv9[:, 0, :, :])
        nc.gpsimd.tensor_mul(gtmp[:], xc[:, 0:16, 2:2 + W], wv9[:, 2, :, :])
        nc.gpsimd.tensor_add(gacc[:], gacc[:], gtmp[:])
        first = True
        for k in korder:
            di, dj = k // 3, k % 3
            xin = xc[:, di:di + 16, dj:dj + W]
            if first:
                nc.vector.tensor_mul(acc[:], xin, wv9[:, k, :, :])
                first = False
            else:
                tmp = pool_sten.tile([96, 16, W], bf16, tag="tmp")
                nc.vector.tensor_mul(tmp[:], xin, wv9[:, k, :, :])
                nc.vector.tensor_add(acc[:], acc[:], tmp[:])
        nc.vector.tensor_add(acc[:], acc[:], gacc[:])
        nc.vector.tensor_mul(xn[:, 1:17, 1:65], acc[:], rSb[:])
        if step < STEPS - 1:
            nc.sync.dma_start(xn[0:72, 17, :], xn[24:96, 1, :])
            nc.scalar.dma_start(xn[24:96, 0, :], xn[0:72, 16, :])
        xc, xn = xn, xc

    es_sten.close()

    # ---------------- decoder ----------------
    for b in range(4):
        (nc.sync if b % 2 == 0 else nc.scalar).dma_start(
            eB[:, 1 + b * 16:17 + b * 16, :],
            xc[b * LD:(b + 1) * LD, 1:17, :])
    U = unfold(eB_f)
    conv_packed(U, 2, eA, cb_t[:, 3:4], AF.Relu)
    U = unfold(eA_f)
    conv_packed(U, 3, eB, cb_t[:, 4:5], AF.Relu)
    U = unfold(eB_f)
    conv_packed(U, 4, None, cb_t[0:1, 5:6], AF.Identity, m=1)

    es_conv.close()
    es_unf.close()
    es_mid.close()

    # ---------------- final MLP (Taylor-in-s polynomial, K=7) ----------------
    # out[i,p,:] = C_i + s_p*B_i + s_p^2*A2_i + s_p^3*A3_i  with bf16 hi/lo
    # splits: sP rows [1, 1, s_hi, s_hi, s_lo, s2, s3] pair with
    # R rows [C_hi, C_lo, B_hi, B_lo, B_hi, A2, A3].
    pool_fin = es.enter_context(tc.tile_pool(name="fin", bufs=1))
    pool_stage = es.enter_context(tc.tile_pool(name="stage", bufs=6))
    ppF = es.enter_context(tc.tile_pool(name="ppF", bufs=4, space="PSUM"))

    s16 = pool_fin.tile([16, 256], f32)
    nc.sync.dma_start(s16[:], s_row[:])
    sh16 = pool_fin.tile([16, 256], bf16)
    nc.vector.tensor_copy(sh16[:], s16[:])
    shf = pool_fin.tile([16, 256], f32)
    nc.vector.tensor_copy(shf[:], sh16[:])
    sl16 = pool_fin.tile([16, 256], bf16)
    nc.vector.tensor_sub(sl16[:], s16[:], shf[:])
    s2f = pool_fin.tile([16, 256], f32)
    nc.vector.tensor_mul(s2f[:], s16[:], s16[:])
    s2_16 = pool_fin.tile([16, 256], bf16)
    nc.vector.tensor_copy(s2_16[:], s2f[:])
    s3_16 = pool_fin.tile([16, 256], bf16)
    nc.vector.tensor_mul(s3_16[:], s2f[:], s16[:])

    sP = pool_fin.tile([7, HW], bf16)
    nc.vector.memset(sP[0:2, :], 1.0)
    nc.sync.dma_start(sP[2:3, :], sh16[:])
    nc.sync.dma_start(sP[3:4, :], sh16[:])
    nc.sync.dma_start(sP[4:5, :], sl16[:])
    nc.sync.dma_start(sP[5:6, :], s2_16[:])
    nc.sync.dma_start(sP[6:7, :], s3_16[:])

    fp16 = mybir.dt.float16
    for i in range(DEPTH):
        for pc2 in range(16):
            stage = pool_stage.tile([128, 2 * ED], fp16, tag="stage")
            for h in range(2):
                pc = pc2 * 2 + h
                pf = ppF.tile([128, ED], f32, tag="pf")
                lhsT = sP[:, pc * 128:(pc + 1) * 128]
                nc.tensor.matmul(pf[:, 0:512], lhsT, R_ts[i][:, 0:512],
                                 start=True, stop=True)
                nc.tensor.matmul(pf[:, 512:ED], lhsT, R_ts[i][:, 512:ED],
                                 start=True, stop=True)
                if h == 0:
                    nc.vector.tensor_copy(stage[:, 0:ED], pf[:])
                else:
                    nc.scalar.copy(stage[:, ED:2 * ED], pf[:])
            eng = nc.sync if pc2 % 2 == 0 else nc.scalar
            eng.dma_start(
                out_d[i, pc2 * 256:(pc2 + 1) * 256, :].rearrange(
                    "(h p) e -> p h e", h=2),
                stage[:].rearrange("p (h e) -> p h e", h=2))


# ---------------------------------------------------------------- host side
def _prep_params(inputs):
    g = {k: np.asarray(v, np.float32) for k, v in inputs.items()}
    perm = np.array([(o % LD) * KK + o // LD for o in range(OC)])  # o'=k*24+l -> l*9+k
    p_reg = g["reg_W"][perm]          # (216, 768) k-major rows
    p_regb_full = g["reg_b"][perm]
    regb = np.zeros((128, 2), np.float32)
    regb[:, 0] = p_regb_full[0:128]
    regb[0:88, 1] = p_regb_full[128:OC]
    ind = np.zeros((OC, LD), np.float32)
    for o in range(OC):
        ind[o, o % LD] = 1.0

    def packK(Wk):  # (O, Cin, 3, 3) -> (9*Cin, O): row k*Cin+cin
        O, Cin = Wk.shape[0], Wk.shape[1]
        out = np.zeros((KK * Cin, O), np.float32)
        for k in range(KK):
            out[k * Cin:(k + 1) * Cin, :] = Wk[:, :, k // 3, k % 3].T
        return out

    cw0 = g["enc_W0"][:, 0, :, :].reshape(LD, KK).T.copy()  # (9, 24)
    cb = np.zeros((LD, 8), np.float32)
    cb[:, 0] = g["enc_b0"]
    cb[:, 1] = g["enc_b1"]
    cb[:, 2] = g["enc_b2"]
    cb[:, 3] = g["dec_b0"]
    cb[:, 4] = g["dec_b1"]
    cb[0, 5] = g["dec_b2"][0]
    cb[:, 6] = EPS

    u = g["lmlp_W"] @ g["da_W"][:, 0]            # (4, 384)
    c = g["lmlp_W"] @ g["da_b"] + g["lmlp_b"]    # (4, 384)
    # Taylor-in-s collapse of gelu(s*u + c) @ sm_W.T + sm_b (|s*u| ~< 1e-4,
    # cubic truncation error ~1e-12): per-layer 768-vec coefficients.
    # sP rows [1, 1, s_hi, s_hi, s_lo, s2, s3] pair with
    # R  rows [C_hi, C_lo, B_hi, B_lo, B_hi, A2, A3].
    from scipy.special import erf as _erf
    Phi = lambda x: 0.5 * (1.0 + _erf(x / np.sqrt(2.0)))
    phi = lambda x: np.exp(-x * x / 2.0) / np.sqrt(2.0 * np.pi)
    smT64 = g["sm_W"].T.astype(np.float64)
    bf = ml_dtypes.bfloat16
    R = np.zeros((4 * 7, ED), np.float32)
    for i in range(DEPTH):
        cj = c[i].astype(np.float64)
        uj = u[i].astype(np.float64)
        g0 = cj * Phi(cj)
        g1 = (Phi(cj) + cj * phi(cj)) * uj
        g2 = 0.5 * phi(cj) * (2.0 - cj ** 2) * uj ** 2
        g3 = (1.0 / 6.0) * phi(cj) * (cj ** 3 - 4.0 * cj) * uj ** 3
        C = (g0 @ smT64 + g["sm_b"]).astype(np.float32)
        B = (g1 @ smT64).astype(np.float32)
        A2 = (g2 @ smT64).astype(np.float32)
        A3 = (g3 @ smT64).astype(np.float32)
        Ch = C.astype(bf).astype(np.float32)
        Bh = B.astype(bf).astype(np.float32)
        R[i * 7 + 0] = Ch
        R[i * 7 + 1] = C - Ch
        R[i * 7 + 2] = Bh
        R[i * 7 + 3] = B - Bh
        R[i * 7 + 4] = Bh
        R[i * 7 + 5] = A2
        R[i * 7 + 6] = A3

    return {
        "p_regT": p_reg.T.astype(bf).copy(),
        "p_regb": regb,
        "p_ind": ind.astype(bf),
        "p_cw0": cw0.astype(bf),
        "p_cwe1": packK(g["enc_W1"]).astype(bf),
        "p_cwe2": packK(g["enc_W2"]).astype(bf),
        "p_cwd0": packK(g["dec_W0"]).astype(bf),
        "p_cwd1": packK(g["dec_W1"]).astype(bf),
        "p_cwd2": packK(g["dec_W2"]).astype(bf),
        "p_cb": cb,
        "p_R": R.astype(bf),
    }


_NC_CACHE = {}


def _get_nc(gelu=True):
    if gelu not in _NC_CACHE:
        _NC_CACHE[gelu] = build_nc(gelu=gelu)
    return _NC_CACHE[gelu]


def run(inputs, trace=False, gelu=True):
    nc = _get_nc(gelu)
    params = _prep_params(inputs)
    depth = np.asarray(inputs["depth"], np.float32)
    cues = np.asarray(inputs["cues"], np.float32)
    in_maps = []
    for n in range(NCORES):
        m = dict(params)
        m["depth"] = np.ascontiguousarray(depth[n].reshape(ED, HW))
        m["cues"] = np.ascontiguousarray(cues[n].reshape(1, HW))
        in_maps.append(m)
    res = run_bass_kernel_spmd(nc, in_maps, list(range(NCORES)), trace=trace)
    out = np.stack([res.results[n]["out"] for n in range(NCORES)], axis=1)
    return out.astype(np.float32), res


def kernel(**inputs):
    out, _ = run(inputs, trace=False)
    return out

